# revision 19
# baseline (speedup 1.0000x reference)
"""Causal self-attention (B=4, T=2048, C=1024, H=16) on 8 TRN2 NeuronCores.

Sharding: tensor-parallel over heads. Each core owns 2 heads:
  Launch A (per core): QKV^T projection for its 2 heads (fp32r matmuls),
    flash-style causal attention with softmax computed in the S^T layout
    (k on partitions, q on free dim; rowsums via a ones-column in V'),
    normalized per-head output y_heads^T [B, 128, T].
  Host: concatenate the 8 per-core head outputs (pure gather) and re-shard
    by token slices.
  Launch B (per core): c_proj for a 1024-token slice (full C contraction)
    + bias -> final [tokens, C] slice. Host concatenates slices.

All matmuls use fp32r (tf32-like, ~1.5e-4 rel err per 128-contraction,
1 cycle/row on the PE for moving dim >= 256). No host FLOPs: the host only
transposes/slices/concatenates.
"""

import os
import time
from contextlib import ExitStack

import numpy as np

import concourse.bass as bass
import concourse.tile as tile
from concourse import bacc, mybir
from concourse.bass_utils import run_bass_kernel_spmd
from concourse.masks import make_identity

B, T, C = 4, 2048, 1024
H, D = 16, 64
NCORES = 8
HPC = H // NCORES            # heads per core = 2
HD = HPC * D                 # per-core head feature width = 128
F32 = mybir.dt.float32
F32R = mybir.dt.float32r

QT = 512                     # q tile (moving free dim)
KT = 128                     # k tile (S^T partition dim)
NQT = T // QT                # 4
NKT = T // KT                # 16

_CACHE = {}

TRACE = os.environ.get("KERNEL_TRACE", "0") == "1"
LAST_EXEC_NS = {}


def _build_launch_a():
    nc = bacc.Bacc("TRN2", target_bir_lowering=False, debug=False)

    xt_d = nc.dram_tensor("xt", [B, C, T], F32R, kind="ExternalInput").ap()
    w_d = nc.dram_tensor("wqkv", [C, 3 * HD], F32R, kind="ExternalInput").ap()
    b_d = nc.dram_tensor("bqkv", [3 * HD], F32, kind="ExternalInput").ap()
    yt_d = nc.dram_tensor("yt", [B, HD, T], F32, kind="ExternalOutput").ap()

    with tile.TileContext(nc) as tc, ExitStack() as ctx:
        consts = ctx.enter_context(tc.tile_pool(name="consts", bufs=1))
        xt_pool = ctx.enter_context(tc.tile_pool(name="xt", bufs=5))
        qkvt_pool = ctx.enter_context(tc.tile_pool(name="qkvt", bufs=2))
        vn_pool = ctx.enter_context(tc.tile_pool(name="vn", bufs=2))
        es_pool = ctx.enter_context(tc.tile_pool(name="es", bufs=6))
        y_pool = ctx.enter_context(tc.tile_pool(name="y", bufs=2))
        small = ctx.enter_context(tc.tile_pool(name="small", bufs=2))
        # PSUM: 8 banks = psS (2x2 banks, S tiles) + psO (2, O accumulators)
        #       + psQ (2, shared by QKV accum / V-transpose / bcast)
        psQ = ctx.enter_context(tc.tile_pool(name="psQ", bufs=2, space="PSUM"))
        psO = ctx.enter_context(tc.tile_pool(name="psO", bufs=2, space="PSUM"))
        psS = ctx.enter_context(tc.tile_pool(name="psS", bufs=2, space="PSUM"))

        # --- constants ---
        w_sb = consts.tile([128, 8, 3 * HD], F32R)     # [p, ct, f]
        nc.sync.dma_start(w_sb[:], w_d.rearrange("(ct p) f -> p ct f", p=128))
        b_sb = consts.tile([128, 3], F32)              # per-partition bias per ftile
        nc.sync.dma_start(b_sb[:], b_d.rearrange("(ft p) -> p ft", p=128))
        ident_f = consts.tile([128, 128], F32)
        make_identity(nc, ident_f[:])
        ident = consts.tile([128, 128], F32R)
        nc.vector.tensor_copy(ident[:], ident_f[:])
        ones64_f = consts.tile([1, 64], F32)
        nc.vector.memset(ones64_f[:], 1.0)
        ones64 = consts.tile([1, 64], F32R)
        nc.vector.tensor_copy(ones64[:], ones64_f[:])
        onescol_f = consts.tile([128, NKT], F32)
        nc.vector.memset(onescol_f[:], 1.0)

        def emit_xt_loads(b):
            # pair two c-tiles per DMA (2 MB each) to amortize fixed DMA cost;
            # xts[ct] below are AP views into the paired tiles
            xts = []
            for g in range(4):
                t = xt_pool.tile([128, 2, T], F32R, tag="xt", name=f"xt{b}_{g}")
                nc.sync.dma_start(
                    t[:],
                    xt_d[b, g * 256:(g + 1) * 256, :].rearrange(
                        "(g2 p) t -> p g2 t", p=128),
                )
                xts.append(t[:, 0, :])
                xts.append(t[:, 1, :])
            return xts

        def qkv_units(b, xts):
            """QKV^T for batch b as 6 schedulable units (ft x token-half)."""
            qkvt = qkvt_pool.tile([128, 3, T], F32R, tag="qkvt", name=f"qkvt{b}")
            units = []
            for ft in range(3):
                for half in range(2):
                    def u(ft=ft, half=half):
                        pss = [psQ.tile([128, QT], F32, tag="psQ",
                                        name=f"qps{b}_{ft}_{half}_{_i}")
                               for _i in range(2)]
                        for ct in range(8):
                            lhsT = w_sb[:, ct, ft * 128:(ft + 1) * 128]
                            for ti in range(2):
                                tt = half * 2 + ti
                                nc.tensor.matmul(
                                    pss[ti][:], lhsT,
                                    xts[ct][:, tt * QT:(tt + 1) * QT],
                                    start=(ct == 0), stop=(ct == 7),
                                )
                        for ti in range(2):
                            tt = half * 2 + ti
                            nc.vector.tensor_scalar_add(
                                qkvt[:, ft, tt * QT:(tt + 1) * QT], pss[ti][:],
                                b_sb[:, ft:ft + 1],
                            )
                    units.append(u)
            return qkvt, units

        def emit_attention(b, qkvt, fill_queue):
            """Attention for batch b; pops interleaved fill work (next batch's
            QKV units) between score/output tile pairs to keep the PE busy
            while ACT runs the exps."""
            vn = vn_pool.tile([128, NKT, 130], F32R, tag="vn", name=f"vn{b}")
            nc.vector.tensor_copy(vn[:, :, 64], onescol_f[:])
            nc.vector.tensor_copy(vn[:, :, 129], onescol_f[:])
            y_sb = y_pool.tile([HD, T], F32, tag="y", name=f"ysb{b}")

            total_pairs = HPC * sum(2 * (qi + 1) for qi in range(NQT))  # 40
            pop_every = max(1, total_pairs // max(1, len(fill_queue)))
            pcount = 0
            tr_done = 0
            for qi in range(NQT):
                nkt = 4 * (qi + 1)
                q0 = qi * QT
                # lazy V-natural transposes for the k-tiles this qi introduces
                for kt in range(tr_done, nkt):
                    trp = psQ.tile([128, 128], F32R, tag="psQ", name=f"trp{b}_{kt}")
                    nc.tensor.transpose(
                        trp[:], qkvt[:, 2, kt * 128:(kt + 1) * 128], ident[:])
                    nc.vector.tensor_copy(vn[:, kt, 0:64], trp[:, 0:64])
                    nc.vector.tensor_copy(vn[:, kt, 65:129], trp[:, 64:128])
                tr_done = nkt
                # per k-tile: (qoff, width, is_diag). Diagonal tiles only need
                # q columns >= (kt*128 - q0): the rest is fully masked.
                plan = []
                for kt in range(nkt):
                    j = kt - 4 * qi
                    if j >= 0:
                        plan.append((kt, j * 128, QT - j * 128, True))
                    else:
                        plan.append((kt, 0, QT, False))
                for h in range(HPC):
                    hp = slice(h * 64, (h + 1) * 64)
                    o_ps = psO.tile([65, QT], F32, tag="psO", name=f"ops{b}_{qi}_{h}")
                    for pi in range(0, nkt, 2):
                        pair = plan[pi:pi + 2]
                        s_ps = psS.tile([128, 1024], F32, tag="psS",
                                        name=f"sps{b}_{qi}_{h}_{pi}")
                        es = es_pool.tile([128, 1024], F32R, tag="es",
                                          name=f"es{b}_{qi}_{h}_{pi}")
                        # pack the pair contiguously, each within one PSUM bank
                        offs, cur = [], 0
                        for (kt, qoff, w, diag) in pair:
                            off = cur if cur + w <= 512 else 512
                            offs.append(off)
                            cur = off + w
                        span = cur
                        for (kt, qoff, w, diag), off in zip(pair, offs):
                            nc.tensor.matmul(
                                s_ps[:, off:off + w],
                                qkvt[hp, 1, kt * 128:(kt + 1) * 128],
                                qkvt[hp, 0, q0 + qoff:q0 + qoff + w],
                                start=True, stop=True,
                            )
                        nc.scalar.activation(
                            out=es[:, 0:span], in_=s_ps[:, 0:span],
                            func=mybir.ActivationFunctionType.Exp, scale=0.125,
                        )
                        for (kt, qoff, w, diag), off in zip(pair, offs):
                            if diag:
                                # zero where local q < p (strict upper triangle)
                                nc.gpsimd.affine_select(
                                    out=es[:, off:off + w],
                                    in_=es[:, off:off + w],
                                    compare_op=mybir.AluOpType.is_ge,
                                    fill=0.0,
                                    base=0,
                                    pattern=[[1, w]],
                                    channel_multiplier=-1,
                                )
                        for (kt, qoff, w, diag), off in zip(pair, offs):
                            nc.tensor.matmul(
                                o_ps[:, qoff:qoff + w],
                                vn[:, kt, h * 65:(h + 1) * 65],
                                es[:, off:off + w],
                                start=(kt == 0), stop=(kt == nkt - 1),
                            )
                        pcount += 1
                        if pcount % pop_every == 0 and fill_queue:
                            fill_queue.pop(0)()
                    # normalize: y = O_unnorm / rowsum (broadcast via K=1 matmul)
                    rcp = small.tile([1, QT], F32R, tag="rcp")
                    with nc.allow_low_precision(reason="f32r is bit-identical to f32"):
                        nc.vector.reciprocal(rcp[:], o_ps[64:65, :])
                    bc_ps = psQ.tile([64, QT], F32, tag="psQ", name=f"bcps{b}_{qi}_{h}")
                    nc.tensor.matmul(bc_ps[:], ones64[:], rcp[:], start=True, stop=True)
                    bc_sb = small.tile([64, QT], F32, tag="bc", name=f"bcsb{b}_{qi}_{h}")
                    nc.vector.tensor_copy(bc_sb[:], bc_ps[:])
                    nc.vector.tensor_mul(y_sb[hp, q0:q0 + QT], o_ps[0:64, :], bc_sb[:])
            while fill_queue:
                fill_queue.pop(0)()
            nc.sync.dma_start(yt_d[b], y_sb[:])

        # software pipeline across batches: QKV(b+1) fills attention(b) gaps
        xts0 = emit_xt_loads(0)
        qkvt_cur, units0 = qkv_units(0, xts0)
        for u in units0:
            u()
        for b in range(B):
            if b + 1 < B:
                xts_n = emit_xt_loads(b + 1)
                qkvt_next, fill = qkv_units(b + 1, xts_n)
            else:
                qkvt_next, fill = None, []
            emit_attention(b, qkvt_cur, fill)
            qkvt_cur = qkvt_next

    nc.compile()
    return nc


def _build_launch_b():
    nc = bacc.Bacc("TRN2", target_bir_lowering=False, debug=False)

    TB = B * T // NCORES     # 1024 tokens per core
    yt_d = nc.dram_tensor("ytc", [C, TB], F32R, kind="ExternalInput").ap()
    w_d = nc.dram_tensor("wp", [C, C], F32R, kind="ExternalInput").ap()
    b_d = nc.dram_tensor("bp", [C], F32, kind="ExternalInput").ap()
    o_d = nc.dram_tensor("out", [TB, C], F32, kind="ExternalOutput").ap()

    with tile.TileContext(nc) as tc, ExitStack() as ctx:
        consts = ctx.enter_context(tc.tile_pool(name="consts", bufs=1))
        wpool = ctx.enter_context(tc.tile_pool(name="wpool", bufs=8))
        ypool = ctx.enter_context(tc.tile_pool(name="ypool", bufs=8))
        outp = ctx.enter_context(tc.tile_pool(name="outp", bufs=4))
        ps = ctx.enter_context(tc.tile_pool(name="ps", bufs=8, space="PSUM"))

        # per-ct tiles so accumulation can start as soon as ct=0 lands
        wts, yts = [], []
        for ct in range(8):
            wt = wpool.tile([128, C], F32R, tag="w", name=f"wt{ct}")
            nc.sync.dma_start(wt[:], w_d[ct * 128:(ct + 1) * 128, :])
            yt = ypool.tile([128, TB], F32R, tag="y", name=f"yct{ct}")
            nc.sync.dma_start(yt[:], yt_d[ct * 128:(ct + 1) * 128, :])
            wts.append(wt)
            yts.append(yt)
        bias = consts.tile([128, C], F32)
        nc.gpsimd.dma_start(
            out=bias[:], in_=bass.AP(tensor=b_d.tensor, offset=0, ap=[[0, 128], [1, C]])
        )

        for half in range(2):
            pss = [[ps.tile([128, 512], F32, tag="ps", name=f"prps{half}_{_m}_{_n}")
                    for _n in range(2)] for _m in range(4)]
            for ct in range(8):
                for mi in range(4):
                    m = half * 4 + mi
                    lhsT = yts[ct][:, m * 128:(m + 1) * 128]
                    for n in range(2):
                        nc.tensor.matmul(
                            pss[mi][n][:], lhsT, wts[ct][:, n * 512:(n + 1) * 512],
                            start=(ct == 0), stop=(ct == 7),
                        )
            for mi in range(4):
                m = half * 4 + mi
                o_sb = outp.tile([128, C], F32, tag="o")
                for n in range(2):
                    nc.vector.tensor_add(
                        o_sb[:, n * 512:(n + 1) * 512], pss[mi][n][:],
                        bias[:, n * 512:(n + 1) * 512],
                    )
                nc.sync.dma_start(o_d[m * 128:(m + 1) * 128, :], o_sb[:])

    nc.compile()
    return nc


def kernel(x, W_attn, b_attn, W_proj, b_proj):
    x = np.asarray(x, dtype=np.float32)
    W_attn = np.asarray(W_attn, dtype=np.float32)
    b_attn = np.asarray(b_attn, dtype=np.float32)
    W_proj = np.asarray(W_proj, dtype=np.float32)
    b_proj = np.asarray(b_proj, dtype=np.float32)

    if "a" not in _CACHE:
        _CACHE["a"] = _build_launch_a()
    if "b" not in _CACHE:
        _CACHE["b"] = _build_launch_b()

    # ---- host prep: transpose/slice only (no FLOPs) ----
    xt = np.ascontiguousarray(x.transpose(0, 2, 1))          # [B, C, T]

    in_a = []
    for c in range(NCORES):
        sl = slice(c * HD, (c + 1) * HD)
        w = np.ascontiguousarray(
            np.concatenate(
                [W_attn[:, sl], W_attn[:, C:][:, sl], W_attn[:, 2 * C:][:, sl]],
                axis=1,
            )
        )
        bq = np.concatenate([b_attn[sl], b_attn[C:][sl], b_attn[2 * C:][sl]])
        in_a.append({"xt": xt, "wqkv": w, "bqkv": np.ascontiguousarray(bq)})

    t0 = time.time()
    ra = run_bass_kernel_spmd(_CACHE["a"], in_a, core_ids=list(range(NCORES)))
    LAST_EXEC_NS["a_wall"] = int((time.time() - t0) * 1e9)
    yts = [r["yt"] for r in ra.results]                      # each [B, HD, T]
    ytf = np.concatenate(yts, axis=1)                        # [B, C, T]

    in_b = []
    for c in range(NCORES):
        bidx, thalf = c // 2, c % 2
        ytc = np.ascontiguousarray(ytf[bidx, :, thalf * 1024:(thalf + 1) * 1024])
        in_b.append({"ytc": ytc, "wp": W_proj, "bp": b_proj})

    t0 = time.time()
    rb = run_bass_kernel_spmd(_CACHE["b"], in_b, core_ids=list(range(NCORES)))
    LAST_EXEC_NS["b_wall"] = int((time.time() - t0) * 1e9)

    out = np.empty((B, T, C), dtype=np.float32)
    for c in range(NCORES):
        bidx, thalf = c // 2, c % 2
        out[bidx, thalf * 1024:(thalf + 1) * 1024, :] = rb.results[c]["out"]
    return out


# revision 21
# speedup vs baseline: 19.6945x; 19.6945x over previous
"""Causal self-attention (B=4, T=2048, C=1024, H=16) on 8 TRN2 NeuronCores.

Sharding: tensor-parallel over heads. Each core owns 2 heads:
  Launch A (per core): QKV^T projection for its 2 heads (fp32r matmuls),
    flash-style causal attention with softmax computed in the S^T layout
    (k on partitions, q on free dim; rowsums via a ones-column in V'),
    normalized per-head output y_heads^T [B, 128, T].
  Host: concatenate the 8 per-core head outputs (pure gather) and re-shard
    by token slices.
  Launch B (per core): c_proj for a 1024-token slice (full C contraction)
    + bias -> final [tokens, C] slice. Host concatenates slices.

All matmuls use fp32r (tf32-like, ~1.5e-4 rel err per 128-contraction,
1 cycle/row on the PE for moving dim >= 256). No host FLOPs: the host only
transposes/slices/concatenates.
"""

import os
import time
from contextlib import ExitStack

import numpy as np

import concourse.bass as bass
import concourse.tile as tile
from concourse import bacc, mybir
from concourse.bass_utils import run_bass_kernel_spmd
from concourse.masks import make_identity

B, T, C = 4, 2048, 1024
H, D = 16, 64
NCORES = 8
HPC = H // NCORES            # heads per core = 2
HD = HPC * D                 # per-core head feature width = 128
F32 = mybir.dt.float32
F32R = mybir.dt.float32r

QT = 512                     # q tile (moving free dim)
KT = 128                     # k tile (S^T partition dim)
NQT = T // QT                # 4
NKT = T // KT                # 16

_CACHE = {}

TRACE = os.environ.get("KERNEL_TRACE", "0") == "1"
LAST_EXEC_NS = {}


def _build_launch_a():
    nc = bacc.Bacc("TRN2", target_bir_lowering=False, debug=False)

    xt_d = nc.dram_tensor("xt", [B, C, T], F32R, kind="ExternalInput").ap()
    w_d = nc.dram_tensor("wqkv", [C, 3 * HD], F32R, kind="ExternalInput").ap()
    b_d = nc.dram_tensor("bqkv", [3 * HD], F32, kind="ExternalInput").ap()
    yt_d = nc.dram_tensor("yt", [B, HD, T], F32, kind="ExternalOutput").ap()

    with tile.TileContext(nc) as tc, ExitStack() as ctx:
        consts = ctx.enter_context(tc.tile_pool(name="consts", bufs=1))
        xt_pool = ctx.enter_context(tc.tile_pool(name="xt", bufs=9))
        qkvt_pool = ctx.enter_context(tc.tile_pool(name="qkvt", bufs=2))
        vn_pool = ctx.enter_context(tc.tile_pool(name="vn", bufs=2))
        es_pool = ctx.enter_context(tc.tile_pool(name="es", bufs=6))
        y_pool = ctx.enter_context(tc.tile_pool(name="y", bufs=2))
        small = ctx.enter_context(tc.tile_pool(name="small", bufs=2))
        # PSUM: 8 banks = psS (2x2 banks, S tiles) + psO (2, O accumulators)
        #       + psQ (2, shared by QKV accum / V-transpose / bcast)
        psQ = ctx.enter_context(tc.tile_pool(name="psQ", bufs=2, space="PSUM"))
        psO = ctx.enter_context(tc.tile_pool(name="psO", bufs=2, space="PSUM"))
        psS = ctx.enter_context(tc.tile_pool(name="psS", bufs=2, space="PSUM"))

        # --- constants ---
        w_sb = consts.tile([128, 8, 3 * HD], F32R)     # [p, ct, f]
        nc.sync.dma_start(w_sb[:], w_d.rearrange("(ct p) f -> p ct f", p=128))
        b_sb = consts.tile([128, 3], F32)              # per-partition bias per ftile
        nc.sync.dma_start(b_sb[:], b_d.rearrange("(ft p) -> p ft", p=128))
        ident_f = consts.tile([128, 128], F32)
        make_identity(nc, ident_f[:])
        ident = consts.tile([128, 128], F32R)
        nc.vector.tensor_copy(ident[:], ident_f[:])
        ones64_f = consts.tile([1, 64], F32)
        nc.vector.memset(ones64_f[:], 1.0)
        ones64 = consts.tile([1, 64], F32R)
        nc.vector.tensor_copy(ones64[:], ones64_f[:])
        onescol_f = consts.tile([128, NKT], F32)
        nc.vector.memset(onescol_f[:], 1.0)

        def emit_xt_loads(b):
            xts = []
            for ct in range(8):
                t = xt_pool.tile([128, T], F32R, tag="xt", name=f"xt{b}_{ct}")
                nc.sync.dma_start(t[:], xt_d[b, ct * 128:(ct + 1) * 128, :])
                xts.append(t)
            return xts

        def qkv_units(b, xts):
            """QKV^T for batch b as 6 schedulable units (ft x token-half)."""
            qkvt = qkvt_pool.tile([128, 3, T], F32R, tag="qkvt", name=f"qkvt{b}")
            units = []
            for ft in range(3):
                for half in range(2):
                    def u(ft=ft, half=half):
                        pss = [psQ.tile([128, QT], F32, tag="psQ",
                                        name=f"qps{b}_{ft}_{half}_{_i}")
                               for _i in range(2)]
                        for ct in range(8):
                            lhsT = w_sb[:, ct, ft * 128:(ft + 1) * 128]
                            for ti in range(2):
                                tt = half * 2 + ti
                                nc.tensor.matmul(
                                    pss[ti][:], lhsT,
                                    xts[ct][:, tt * QT:(tt + 1) * QT],
                                    start=(ct == 0), stop=(ct == 7),
                                )
                        for ti in range(2):
                            tt = half * 2 + ti
                            nc.vector.tensor_scalar_add(
                                qkvt[:, ft, tt * QT:(tt + 1) * QT], pss[ti][:],
                                b_sb[:, ft:ft + 1],
                            )
                    units.append(u)
            return qkvt, units

        def emit_attention(b, qkvt, fill_queue):
            """Attention for batch b; pops interleaved fill work (next batch's
            QKV units) between score/output tile pairs to keep the PE busy
            while ACT runs the exps."""
            vn = vn_pool.tile([128, NKT, 130], F32R, tag="vn", name=f"vn{b}")
            nc.vector.tensor_copy(vn[:, :, 64], onescol_f[:])
            nc.vector.tensor_copy(vn[:, :, 129], onescol_f[:])
            y_sb = y_pool.tile([HD, T], F32, tag="y", name=f"ysb{b}")

            total_pairs = HPC * sum(2 * (qi + 1) for qi in range(NQT))  # 40
            pop_every = max(1, total_pairs // max(1, len(fill_queue)))
            pcount = 0
            tr_done = 0
            for qi in range(NQT):
                nkt = 4 * (qi + 1)
                q0 = qi * QT
                # lazy V-natural transposes for the k-tiles this qi introduces
                for kt in range(tr_done, nkt):
                    trp = psQ.tile([128, 128], F32R, tag="psQ", name=f"trp{b}_{kt}")
                    nc.tensor.transpose(
                        trp[:], qkvt[:, 2, kt * 128:(kt + 1) * 128], ident[:])
                    nc.vector.tensor_copy(vn[:, kt, 0:64], trp[:, 0:64])
                    nc.vector.tensor_copy(vn[:, kt, 65:129], trp[:, 64:128])
                tr_done = nkt
                # per k-tile: (qoff, width, is_diag). Diagonal tiles only need
                # q columns >= (kt*128 - q0): the rest is fully masked.
                plan = []
                for kt in range(nkt):
                    j = kt - 4 * qi
                    if j >= 0:
                        plan.append((kt, j * 128, QT - j * 128, True))
                    else:
                        plan.append((kt, 0, QT, False))
                for h in range(HPC):
                    hp = slice(h * 64, (h + 1) * 64)
                    o_ps = psO.tile([65, QT], F32, tag="psO", name=f"ops{b}_{qi}_{h}")
                    for pi in range(0, nkt, 2):
                        pair = plan[pi:pi + 2]
                        s_ps = psS.tile([128, 1024], F32, tag="psS",
                                        name=f"sps{b}_{qi}_{h}_{pi}")
                        es = es_pool.tile([128, 1024], F32R, tag="es",
                                          name=f"es{b}_{qi}_{h}_{pi}")
                        # pack the pair contiguously, each within one PSUM bank
                        offs, cur = [], 0
                        for (kt, qoff, w, diag) in pair:
                            off = cur if cur + w <= 512 else 512
                            offs.append(off)
                            cur = off + w
                        span = cur
                        for (kt, qoff, w, diag), off in zip(pair, offs):
                            nc.tensor.matmul(
                                s_ps[:, off:off + w],
                                qkvt[hp, 1, kt * 128:(kt + 1) * 128],
                                qkvt[hp, 0, q0 + qoff:q0 + qoff + w],
                                start=True, stop=True,
                            )
                        nc.scalar.activation(
                            out=es[:, 0:span], in_=s_ps[:, 0:span],
                            func=mybir.ActivationFunctionType.Exp, scale=0.125,
                        )
                        for (kt, qoff, w, diag), off in zip(pair, offs):
                            if diag:
                                # zero where local q < p (strict upper triangle)
                                nc.gpsimd.affine_select(
                                    out=es[:, off:off + w],
                                    in_=es[:, off:off + w],
                                    compare_op=mybir.AluOpType.is_ge,
                                    fill=0.0,
                                    base=0,
                                    pattern=[[1, w]],
                                    channel_multiplier=-1,
                                )
                        for (kt, qoff, w, diag), off in zip(pair, offs):
                            nc.tensor.matmul(
                                o_ps[:, qoff:qoff + w],
                                vn[:, kt, h * 65:(h + 1) * 65],
                                es[:, off:off + w],
                                start=(kt == 0), stop=(kt == nkt - 1),
                            )
                        pcount += 1
                        if pcount % pop_every == 0 and fill_queue:
                            fill_queue.pop(0)()
                    # normalize: y = O_unnorm / rowsum (broadcast via K=1 matmul)
                    rcp = small.tile([1, QT], F32R, tag="rcp")
                    with nc.allow_low_precision(reason="f32r is bit-identical to f32"):
                        nc.vector.reciprocal(rcp[:], o_ps[64:65, :])
                    bc_ps = psQ.tile([64, QT], F32, tag="psQ", name=f"bcps{b}_{qi}_{h}")
                    nc.tensor.matmul(bc_ps[:], ones64[:], rcp[:], start=True, stop=True)
                    bc_sb = small.tile([64, QT], F32, tag="bc", name=f"bcsb{b}_{qi}_{h}")
                    nc.vector.tensor_copy(bc_sb[:], bc_ps[:])
                    nc.vector.tensor_mul(y_sb[hp, q0:q0 + QT], o_ps[0:64, :], bc_sb[:])
            while fill_queue:
                fill_queue.pop(0)()
            nc.sync.dma_start(yt_d[b], y_sb[:])

        # software pipeline across batches: QKV(b+1) fills attention(b) gaps
        xts0 = emit_xt_loads(0)
        qkvt_cur, units0 = qkv_units(0, xts0)
        for u in units0:
            u()
        for b in range(B):
            if b + 1 < B:
                xts_n = emit_xt_loads(b + 1)
                qkvt_next, fill = qkv_units(b + 1, xts_n)
            else:
                qkvt_next, fill = None, []
            emit_attention(b, qkvt_cur, fill)
            qkvt_cur = qkvt_next

    nc.compile()
    return nc


def _build_launch_b():
    nc = bacc.Bacc("TRN2", target_bir_lowering=False, debug=False)

    TB = B * T // NCORES     # 1024 tokens per core
    yt_d = nc.dram_tensor("ytc", [C, TB], F32R, kind="ExternalInput").ap()
    w_d = nc.dram_tensor("wp", [C, C], F32R, kind="ExternalInput").ap()
    b_d = nc.dram_tensor("bp", [C], F32, kind="ExternalInput").ap()
    o_d = nc.dram_tensor("out", [TB, C], F32, kind="ExternalOutput").ap()

    with tile.TileContext(nc) as tc, ExitStack() as ctx:
        consts = ctx.enter_context(tc.tile_pool(name="consts", bufs=1))
        wpool = ctx.enter_context(tc.tile_pool(name="wpool", bufs=8))
        ypool = ctx.enter_context(tc.tile_pool(name="ypool", bufs=8))
        outp = ctx.enter_context(tc.tile_pool(name="outp", bufs=4))
        ps = ctx.enter_context(tc.tile_pool(name="ps", bufs=8, space="PSUM"))

        # per-ct tiles so accumulation can start as soon as ct=0 lands
        wts, yts = [], []
        for ct in range(8):
            wt = wpool.tile([128, C], F32R, tag="w", name=f"wt{ct}")
            nc.sync.dma_start(wt[:], w_d[ct * 128:(ct + 1) * 128, :])
            yt = ypool.tile([128, TB], F32R, tag="y", name=f"yct{ct}")
            nc.sync.dma_start(yt[:], yt_d[ct * 128:(ct + 1) * 128, :])
            wts.append(wt)
            yts.append(yt)
        bias = consts.tile([128, C], F32)
        nc.gpsimd.dma_start(
            out=bias[:], in_=bass.AP(tensor=b_d.tensor, offset=0, ap=[[0, 128], [1, C]])
        )

        for half in range(2):
            pss = [[ps.tile([128, 512], F32, tag="ps", name=f"prps{half}_{_m}_{_n}")
                    for _n in range(2)] for _m in range(4)]
            for ct in range(8):
                for mi in range(4):
                    m = half * 4 + mi
                    lhsT = yts[ct][:, m * 128:(m + 1) * 128]
                    for n in range(2):
                        nc.tensor.matmul(
                            pss[mi][n][:], lhsT, wts[ct][:, n * 512:(n + 1) * 512],
                            start=(ct == 0), stop=(ct == 7),
                        )
            for mi in range(4):
                m = half * 4 + mi
                o_sb = outp.tile([128, C], F32, tag="o")
                for n in range(2):
                    nc.vector.tensor_add(
                        o_sb[:, n * 512:(n + 1) * 512], pss[mi][n][:],
                        bias[:, n * 512:(n + 1) * 512],
                    )
                nc.sync.dma_start(o_d[m * 128:(m + 1) * 128, :], o_sb[:])

    nc.compile()
    return nc


def kernel(x, W_attn, b_attn, W_proj, b_proj):
    x = np.asarray(x, dtype=np.float32)
    W_attn = np.asarray(W_attn, dtype=np.float32)
    b_attn = np.asarray(b_attn, dtype=np.float32)
    W_proj = np.asarray(W_proj, dtype=np.float32)
    b_proj = np.asarray(b_proj, dtype=np.float32)

    if "a" not in _CACHE:
        _CACHE["a"] = _build_launch_a()
    if "b" not in _CACHE:
        _CACHE["b"] = _build_launch_b()

    # ---- host prep: transpose/slice only (no FLOPs) ----
    xt = np.ascontiguousarray(x.transpose(0, 2, 1))          # [B, C, T]

    in_a = []
    for c in range(NCORES):
        sl = slice(c * HD, (c + 1) * HD)
        w = np.ascontiguousarray(
            np.concatenate(
                [W_attn[:, sl], W_attn[:, C:][:, sl], W_attn[:, 2 * C:][:, sl]],
                axis=1,
            )
        )
        bq = np.concatenate([b_attn[sl], b_attn[C:][sl], b_attn[2 * C:][sl]])
        in_a.append({"xt": xt, "wqkv": w, "bqkv": np.ascontiguousarray(bq)})

    t0 = time.time()
    ra = run_bass_kernel_spmd(_CACHE["a"], in_a, core_ids=list(range(NCORES)))
    LAST_EXEC_NS["a_wall"] = int((time.time() - t0) * 1e9)
    yts = [r["yt"] for r in ra.results]                      # each [B, HD, T]
    ytf = np.concatenate(yts, axis=1)                        # [B, C, T]

    in_b = []
    for c in range(NCORES):
        bidx, thalf = c // 2, c % 2
        ytc = np.ascontiguousarray(ytf[bidx, :, thalf * 1024:(thalf + 1) * 1024])
        in_b.append({"ytc": ytc, "wp": W_proj, "bp": b_proj})

    t0 = time.time()
    rb = run_bass_kernel_spmd(_CACHE["b"], in_b, core_ids=list(range(NCORES)))
    LAST_EXEC_NS["b_wall"] = int((time.time() - t0) * 1e9)

    out = np.empty((B, T, C), dtype=np.float32)
    for c in range(NCORES):
        bidx, thalf = c // 2, c % 2
        out[bidx, thalf * 1024:(thalf + 1) * 1024, :] = rb.results[c]["out"]
    return out


# revision 24
# speedup vs baseline: 20.1525x; 1.0233x over previous
"""Causal self-attention (B=4, T=2048, C=1024, H=16) on 8 TRN2 NeuronCores.

Sharding: tensor-parallel over heads. Each core owns 2 heads:
  Launch A (per core): QKV^T projection for its 2 heads (fp32r matmuls),
    flash-style causal attention with softmax computed in the S^T layout
    (k on partitions, q on free dim; rowsums via a ones-column in V'),
    normalized per-head output y_heads^T [B, 128, T].
  Host: concatenate the 8 per-core head outputs (pure gather) and re-shard
    by token slices.
  Launch B (per core): c_proj for a 1024-token slice (full C contraction)
    + bias -> final [tokens, C] slice. Host concatenates slices.

All matmuls use fp32r (tf32-like, ~1.5e-4 rel err per 128-contraction,
1 cycle/row on the PE for moving dim >= 256). No host FLOPs: the host only
transposes/slices/concatenates.
"""

import os
import time
from contextlib import ExitStack

import numpy as np

import concourse.bass as bass
import concourse.tile as tile
from concourse import bacc, mybir
from concourse.bass_utils import run_bass_kernel_spmd
from concourse.masks import make_identity

B, T, C = 4, 2048, 1024
H, D = 16, 64
NCORES = 8
HPC = H // NCORES            # heads per core = 2
HD = HPC * D                 # per-core head feature width = 128
F32 = mybir.dt.float32
F32R = mybir.dt.float32r

QT = 512                     # q tile (moving free dim)
KT = 128                     # k tile (S^T partition dim)
NQT = T // QT                # 4
NKT = T // KT                # 16

_CACHE = {}

TRACE = os.environ.get("KERNEL_TRACE", "0") == "1"
LAST_EXEC_NS = {}


def _build_launch_a():
    nc = bacc.Bacc("TRN2", target_bir_lowering=False, debug=False)

    xt_d = nc.dram_tensor("xt", [B, C, T], F32R, kind="ExternalInput").ap()
    w_d = nc.dram_tensor("wqkv", [C, 3 * HD], F32R, kind="ExternalInput").ap()
    b_d = nc.dram_tensor("bqkv", [3 * HD], F32, kind="ExternalInput").ap()
    yt_d = nc.dram_tensor("yt", [B, HD, T], F32, kind="ExternalOutput").ap()

    with tile.TileContext(nc) as tc, ExitStack() as ctx:
        consts = ctx.enter_context(tc.tile_pool(name="consts", bufs=1))
        xt_pool = ctx.enter_context(tc.tile_pool(name="xt", bufs=9))
        qkvt_pool = ctx.enter_context(tc.tile_pool(name="qkvt", bufs=2))
        vn_pool = ctx.enter_context(tc.tile_pool(name="vn", bufs=2))
        es_pool = ctx.enter_context(tc.tile_pool(name="es", bufs=6))
        y_pool = ctx.enter_context(tc.tile_pool(name="y", bufs=2))
        small = ctx.enter_context(tc.tile_pool(name="small", bufs=2))
        # PSUM: 8 banks = psS (2x2 banks, S tiles) + psO (2, O accumulators)
        #       + psQ (2, shared by QKV accum / V-transpose / bcast)
        psQ = ctx.enter_context(tc.tile_pool(name="psQ", bufs=2, space="PSUM"))
        psO = ctx.enter_context(tc.tile_pool(name="psO", bufs=2, space="PSUM"))
        psS = ctx.enter_context(tc.tile_pool(name="psS", bufs=2, space="PSUM"))

        # --- constants ---
        w_sb = consts.tile([128, 8, 3 * HD], F32R)     # [p, ct, f]
        nc.sync.dma_start(w_sb[:], w_d.rearrange("(ct p) f -> p ct f", p=128))
        b_sb = consts.tile([128, 3], F32)              # per-partition bias per ftile
        nc.sync.dma_start(b_sb[:], b_d.rearrange("(ft p) -> p ft", p=128))
        ident_f = consts.tile([128, 128], F32)
        make_identity(nc, ident_f[:])
        ident = consts.tile([128, 128], F32R)
        nc.vector.tensor_copy(ident[:], ident_f[:])
        ones64_f = consts.tile([1, 64], F32)
        nc.vector.memset(ones64_f[:], 1.0)
        ones64 = consts.tile([1, 64], F32R)
        nc.vector.tensor_copy(ones64[:], ones64_f[:])
        onescol_f = consts.tile([128, NKT], F32)
        nc.vector.memset(onescol_f[:], 1.0)

        def emit_xt_loads(b):
            xts = []
            for ct in range(8):
                t = xt_pool.tile([128, T], F32R, tag="xt", name=f"xt{b}_{ct}")
                nc.sync.dma_start(t[:], xt_d[b, ct * 128:(ct + 1) * 128, :])
                xts.append(t)
            return xts

        def qkv_units(b, xts):
            """QKV^T for batch b as 6 schedulable units (ft x token-half)."""
            qkvt = qkvt_pool.tile([128, 3, T], F32R, tag="qkvt", name=f"qkvt{b}")
            units = []
            for ft in range(3):
                for half in range(2):
                    def u(ft=ft, half=half):
                        pss = [psQ.tile([128, QT], F32, tag="psQ",
                                        name=f"qps{b}_{ft}_{half}_{_i}")
                               for _i in range(2)]
                        for ct in range(8):
                            lhsT = w_sb[:, ct, ft * 128:(ft + 1) * 128]
                            for ti in range(2):
                                tt = half * 2 + ti
                                nc.tensor.matmul(
                                    pss[ti][:], lhsT,
                                    xts[ct][:, tt * QT:(tt + 1) * QT],
                                    start=(ct == 0), stop=(ct == 7),
                                )
                        for ti in range(2):
                            tt = half * 2 + ti
                            nc.vector.tensor_scalar_add(
                                qkvt[:, ft, tt * QT:(tt + 1) * QT], pss[ti][:],
                                b_sb[:, ft:ft + 1],
                            )
                    units.append(u)
            return qkvt, units

        def emit_attention(b, qkvt, fill_queue):
            """Attention for batch b; pops interleaved fill work (next batch's
            QKV units) between score/output tile pairs to keep the PE busy
            while ACT runs the exps."""
            vn = vn_pool.tile([128, NKT, 130], F32R, tag="vn", name=f"vn{b}")
            nc.vector.tensor_copy(vn[:, :, 64], onescol_f[:])
            nc.vector.tensor_copy(vn[:, :, 129], onescol_f[:])
            y_sb = y_pool.tile([HD, T], F32, tag="y", name=f"ysb{b}")

            total_pairs = HPC * sum(2 * (qi + 1) for qi in range(NQT))  # 40
            pop_every = max(1, total_pairs // max(1, len(fill_queue)))
            pcount = 0
            tr_done = 0
            for qi in range(NQT):
                nkt = 4 * (qi + 1)
                q0 = qi * QT
                # lazy V-natural transposes for the k-tiles this qi introduces
                for kt in range(tr_done, nkt):
                    trp = psQ.tile([128, 128], F32R, tag="psQ", name=f"trp{b}_{kt}")
                    nc.tensor.transpose(
                        trp[:], qkvt[:, 2, kt * 128:(kt + 1) * 128], ident[:])
                    nc.vector.tensor_copy(vn[:, kt, 0:64], trp[:, 0:64])
                    nc.vector.tensor_copy(vn[:, kt, 65:129], trp[:, 64:128])
                tr_done = nkt
                # per k-tile: (qoff, width, is_diag). Diagonal tiles only need
                # q columns >= (kt*128 - q0): the rest is fully masked.
                plan = []
                for kt in range(nkt):
                    j = kt - 4 * qi
                    if j >= 0:
                        plan.append((kt, j * 128, QT - j * 128, True))
                    else:
                        plan.append((kt, 0, QT, False))
                for h in range(HPC):
                    hp = slice(h * 64, (h + 1) * 64)
                    o_ps = psO.tile([65, QT], F32, tag="psO", name=f"ops{b}_{qi}_{h}")
                    for pi in range(0, nkt, 2):
                        pair = plan[pi:pi + 2]
                        s_ps = psS.tile([128, 1024], F32, tag="psS",
                                        name=f"sps{b}_{qi}_{h}_{pi}")
                        es = es_pool.tile([128, 1024], F32R, tag="es",
                                          name=f"es{b}_{qi}_{h}_{pi}")
                        # pack the pair contiguously, each within one PSUM bank
                        offs, cur = [], 0
                        for (kt, qoff, w, diag) in pair:
                            off = cur if cur + w <= 512 else 512
                            offs.append(off)
                            cur = off + w
                        span = cur
                        for (kt, qoff, w, diag), off in zip(pair, offs):
                            nc.tensor.matmul(
                                s_ps[:, off:off + w],
                                qkvt[hp, 1, kt * 128:(kt + 1) * 128],
                                qkvt[hp, 0, q0 + qoff:q0 + qoff + w],
                                start=True, stop=True,
                            )
                        nc.scalar.activation(
                            out=es[:, 0:span], in_=s_ps[:, 0:span],
                            func=mybir.ActivationFunctionType.Exp, scale=0.125,
                        )
                        for (kt, qoff, w, diag), off in zip(pair, offs):
                            if diag:
                                # zero where local q < p (strict upper triangle)
                                nc.gpsimd.affine_select(
                                    out=es[:, off:off + w],
                                    in_=es[:, off:off + w],
                                    compare_op=mybir.AluOpType.is_ge,
                                    fill=0.0,
                                    base=0,
                                    pattern=[[1, w]],
                                    channel_multiplier=-1,
                                )
                        for (kt, qoff, w, diag), off in zip(pair, offs):
                            nc.tensor.matmul(
                                o_ps[:, qoff:qoff + w],
                                vn[:, kt, h * 65:(h + 1) * 65],
                                es[:, off:off + w],
                                start=(kt == 0), stop=(kt == nkt - 1),
                            )
                        pcount += 1
                        if pcount % pop_every == 0 and fill_queue:
                            fill_queue.pop(0)()
                    # normalize: y = O_unnorm / rowsum (broadcast via K=1 matmul)
                    rcp = small.tile([1, QT], F32R, tag="rcp")
                    with nc.allow_low_precision(reason="f32r is bit-identical to f32"):
                        nc.vector.reciprocal(rcp[:], o_ps[64:65, :])
                    bc_ps = psQ.tile([64, QT], F32, tag="psQ", name=f"bcps{b}_{qi}_{h}")
                    nc.tensor.matmul(bc_ps[:], ones64[:], rcp[:], start=True, stop=True)
                    bc_sb = small.tile([64, QT], F32, tag="bc", name=f"bcsb{b}_{qi}_{h}")
                    nc.vector.tensor_copy(bc_sb[:], bc_ps[:])
                    nc.vector.tensor_mul(y_sb[hp, q0:q0 + QT], o_ps[0:64, :], bc_sb[:])
                # both heads' q-slice done: stream it out now
                nc.sync.dma_start(yt_d[b][:, q0:q0 + QT], y_sb[:, q0:q0 + QT])
            while fill_queue:
                fill_queue.pop(0)()

        def emit_qkv0_streamed(xts):
            """Batch-0 QKV with ct-outer accumulation, two passes over token
            halves. Six accumulators (3 ft x 2 token-tiles) live at once in
            the not-yet-used attention PSUM pools, so each arriving xt tile is
            consumed with 6 matmuls at DMA pace instead of stalling on ft=0."""
            qkvt = qkvt_pool.tile([128, 3, T], F32R, tag="qkvt", name="qkvt0")
            for tthalf in range(2):
                sps = psS.tile([128, 1024], F32, tag="psS", name=f"q0s{tthalf}")
                accs = [
                    psQ.tile([128, QT], F32, tag="psQ", name=f"q0q0_{tthalf}")[:],
                    psQ.tile([128, QT], F32, tag="psQ", name=f"q0q1_{tthalf}")[:],
                    psO.tile([128, QT], F32, tag="psO", name=f"q0o0_{tthalf}")[:],
                    psO.tile([128, QT], F32, tag="psO", name=f"q0o1_{tthalf}")[:],
                    sps[:, 0:QT],
                    sps[:, QT:2 * QT],
                ]
                for ct in range(8):
                    for ft in range(3):
                        lhsT = w_sb[:, ct, ft * 128:(ft + 1) * 128]
                        for ti in range(2):
                            tt = tthalf * 2 + ti
                            nc.tensor.matmul(
                                accs[ft * 2 + ti], lhsT,
                                xts[ct][:, tt * QT:(tt + 1) * QT],
                                start=(ct == 0), stop=(ct == 7),
                            )
                for ft in range(3):
                    for ti in range(2):
                        tt = tthalf * 2 + ti
                        nc.vector.tensor_scalar_add(
                            qkvt[:, ft, tt * QT:(tt + 1) * QT], accs[ft * 2 + ti],
                            b_sb[:, ft:ft + 1],
                        )
            return qkvt

        # software pipeline across batches: QKV(b+1) fills attention(b) gaps
        xts0 = emit_xt_loads(0)
        qkvt_cur = emit_qkv0_streamed(xts0)
        for b in range(B):
            if b + 1 < B:
                xts_n = emit_xt_loads(b + 1)
                qkvt_next, fill = qkv_units(b + 1, xts_n)
            else:
                qkvt_next, fill = None, []
            emit_attention(b, qkvt_cur, fill)
            qkvt_cur = qkvt_next

    nc.compile()
    return nc


def _build_launch_b():
    nc = bacc.Bacc("TRN2", target_bir_lowering=False, debug=False)

    TB = B * T // NCORES     # 1024 tokens per core
    yt_d = nc.dram_tensor("ytc", [C, TB], F32R, kind="ExternalInput").ap()
    w_d = nc.dram_tensor("wp", [C, C], F32R, kind="ExternalInput").ap()
    b_d = nc.dram_tensor("bp", [C], F32, kind="ExternalInput").ap()
    o_d = nc.dram_tensor("out", [TB, C], F32, kind="ExternalOutput").ap()

    with tile.TileContext(nc) as tc, ExitStack() as ctx:
        consts = ctx.enter_context(tc.tile_pool(name="consts", bufs=1))
        wpool = ctx.enter_context(tc.tile_pool(name="wpool", bufs=8))
        ypool = ctx.enter_context(tc.tile_pool(name="ypool", bufs=8))
        outp = ctx.enter_context(tc.tile_pool(name="outp", bufs=4))
        ps = ctx.enter_context(tc.tile_pool(name="ps", bufs=8, space="PSUM"))

        # per-ct tiles so accumulation can start as soon as ct=0 lands
        wts, yts = [], []
        for ct in range(8):
            wt = wpool.tile([128, C], F32R, tag="w", name=f"wt{ct}")
            nc.sync.dma_start(wt[:], w_d[ct * 128:(ct + 1) * 128, :])
            yt = ypool.tile([128, TB], F32R, tag="y", name=f"yct{ct}")
            nc.sync.dma_start(yt[:], yt_d[ct * 128:(ct + 1) * 128, :])
            wts.append(wt)
            yts.append(yt)
        bias = consts.tile([128, C], F32)
        nc.gpsimd.dma_start(
            out=bias[:], in_=bass.AP(tensor=b_d.tensor, offset=0, ap=[[0, 128], [1, C]])
        )

        for half in range(2):
            pss = [[ps.tile([128, 512], F32, tag="ps", name=f"prps{half}_{_m}_{_n}")
                    for _n in range(2)] for _m in range(4)]
            for ct in range(8):
                for mi in range(4):
                    m = half * 4 + mi
                    lhsT = yts[ct][:, m * 128:(m + 1) * 128]
                    for n in range(2):
                        nc.tensor.matmul(
                            pss[mi][n][:], lhsT, wts[ct][:, n * 512:(n + 1) * 512],
                            start=(ct == 0), stop=(ct == 7),
                        )
            for mi in range(4):
                m = half * 4 + mi
                o_sb = outp.tile([128, C], F32, tag="o")
                for n in range(2):
                    nc.vector.tensor_add(
                        o_sb[:, n * 512:(n + 1) * 512], pss[mi][n][:],
                        bias[:, n * 512:(n + 1) * 512],
                    )
                nc.sync.dma_start(o_d[m * 128:(m + 1) * 128, :], o_sb[:])

    nc.compile()
    return nc


def kernel(x, W_attn, b_attn, W_proj, b_proj):
    x = np.asarray(x, dtype=np.float32)
    W_attn = np.asarray(W_attn, dtype=np.float32)
    b_attn = np.asarray(b_attn, dtype=np.float32)
    W_proj = np.asarray(W_proj, dtype=np.float32)
    b_proj = np.asarray(b_proj, dtype=np.float32)

    if "a" not in _CACHE:
        _CACHE["a"] = _build_launch_a()
    if "b" not in _CACHE:
        _CACHE["b"] = _build_launch_b()

    # ---- host prep: transpose/slice only (no FLOPs) ----
    xt = np.ascontiguousarray(x.transpose(0, 2, 1))          # [B, C, T]

    in_a = []
    for c in range(NCORES):
        sl = slice(c * HD, (c + 1) * HD)
        w = np.ascontiguousarray(
            np.concatenate(
                [W_attn[:, sl], W_attn[:, C:][:, sl], W_attn[:, 2 * C:][:, sl]],
                axis=1,
            )
        )
        bq = np.concatenate([b_attn[sl], b_attn[C:][sl], b_attn[2 * C:][sl]])
        in_a.append({"xt": xt, "wqkv": w, "bqkv": np.ascontiguousarray(bq)})

    t0 = time.time()
    ra = run_bass_kernel_spmd(_CACHE["a"], in_a, core_ids=list(range(NCORES)))
    LAST_EXEC_NS["a_wall"] = int((time.time() - t0) * 1e9)
    yts = [r["yt"] for r in ra.results]                      # each [B, HD, T]
    ytf = np.concatenate(yts, axis=1)                        # [B, C, T]

    in_b = []
    for c in range(NCORES):
        bidx, thalf = c // 2, c % 2
        ytc = np.ascontiguousarray(ytf[bidx, :, thalf * 1024:(thalf + 1) * 1024])
        in_b.append({"ytc": ytc, "wp": W_proj, "bp": b_proj})

    t0 = time.time()
    rb = run_bass_kernel_spmd(_CACHE["b"], in_b, core_ids=list(range(NCORES)))
    LAST_EXEC_NS["b_wall"] = int((time.time() - t0) * 1e9)

    out = np.empty((B, T, C), dtype=np.float32)
    for c in range(NCORES):
        bidx, thalf = c // 2, c % 2
        out[bidx, thalf * 1024:(thalf + 1) * 1024, :] = rb.results[c]["out"]
    return out
